# revision 4
# baseline (speedup 1.0000x reference)
"""Trainium2 Bass kernel for nn_DivTree (moe_routing) — bf16 + fused L3.

Computation (per reference):
    x1 = relu(x0 @ W_shared + b_shared)         # [B, A, H]
    h  = relu(einsum('bah,ahk', x1, W1[route]) + b1[route])
    y  = einsum('bah,ahk', h, W2[route]) + b2[route]   # [B, A, NA]

Strategy: data-parallel over batch across 8 NeuronCores (512 rows/core),
weights replicated, agents grouped by expert (8 distinct experts).
Feature-major layout for L1/L2: contraction on SBUF partitions, weights
stationary, batch as the 512-wide moving free dim. All matmul operands
bf16 (fp32 PSUM accumulation, fp32 output): same PE stream rate as
f32r, but FWL halves LDWEIGHTS and DMA bytes halve.

v2 trace post-mortem drove two fixes:
  * x0 DMA was issued one agent ahead and landed just-in-time; the
    first matmul of each agent stalled ~300ns on the DMA semaphore.
    Now x0 is prefetched 3 agents ahead (xpool bufs=4).
  * L3 (feature-major, [H=512]->[NA=32]) matmuls stalled on the
    trailing h activations: they were emitted after all four h m-tile
    activations, so each one waited ~300-700ns. L3's k-tile matmul k
    only needs h m-tile k, so it now issues right after L2 m-tile
    k+1's matmuls (a full m-group of slack for the activation), and
    the last k-tile matmul is deferred until after the NEXT agent's
    L1 matmuls. L3 stays feature-major (W2 stationary, 32 columns):
    a batch-major h-stationary form would pay a ~97ns LDWEIGHTS per
    tiny matmul — worse than the 4 x 512-column streams.
"""

import numpy as np

P = 128
N_CORES = 8

_cache: dict = {}


def _build(A, D, H, NA, Bl, groups):
    import concourse.mybir as mybir
    import concourse.tile as tile
    from concourse import bacc
    from contextlib import ExitStack

    f32 = mybir.dt.float32
    bf16 = mybir.dt.bfloat16
    Relu = mybir.ActivationFunctionType.Relu
    E = len(groups)
    KD, KH, MH = D // P, H // P, H // P
    NB = Bl  # matmul free dim (batch); Bl=512 fits one PSUM bank
    JB = NB // P  # batch blocks of 128 for L3 (stationary columns)
    assert NB <= 512 and H % P == 0 and D % P == 0 and NA <= P

    agent_list = [(s, a) for s, agents in enumerate(groups) for a in agents]
    NAG = len(agent_list)

    nc = bacc.Bacc()
    x0t = nc.declare_dram_parameter("x0t", [A, D, Bl], bf16, isOutput=False)
    ws = nc.declare_dram_parameter("ws", [D, H], bf16, isOutput=False)
    bs = nc.declare_dram_parameter("bs", [H], f32, isOutput=False)
    w1g = nc.declare_dram_parameter("w1g", [E, H, H], bf16, isOutput=False)
    b1g = nc.declare_dram_parameter("b1g", [E, H], f32, isOutput=False)
    w2g = nc.declare_dram_parameter("w2g", [E, H, NA], bf16, isOutput=False)
    b2r = nc.declare_dram_parameter("b2r", [E, NA, 1], f32, isOutput=False)
    yt = nc.declare_dram_parameter("yt", [A, NA, Bl], f32, isOutput=True)

    with tile.TileContext(nc) as tc, ExitStack() as ctx:
        const = ctx.enter_context(tc.tile_pool(name="const", bufs=1))
        wpool = ctx.enter_context(tc.tile_pool(name="wexp", bufs=2))
        xpool = ctx.enter_context(tc.tile_pool(name="x0", bufs=4))
        x1pool = ctx.enter_context(tc.tile_pool(name="x1", bufs=3))
        hpool = ctx.enter_context(tc.tile_pool(name="h", bufs=3))
        opool = ctx.enter_context(tc.tile_pool(name="out", bufs=3))
        psum = ctx.enter_context(tc.tile_pool(name="ps", bufs=3, space="PSUM"))
        psum2 = ctx.enter_context(tc.tile_pool(name="ps2", bufs=4, space="PSUM"))
        psum3 = ctx.enter_context(tc.tile_pool(name="ps3", bufs=1, space="PSUM"))

        # PE warm-up: the HAM clock gate holds the array at 1.2GHz until it
        # has been busy ~3.4us. Burn dummy matmuls during the initial DMA
        # wait so the real matmuls start at full clock.
        dummy = const.tile([P, 128], bf16)
        nc.gpsimd.memset(dummy[:], 0.0)
        dps = psum.tile([64, 128], f32, tag="ps")
        for i in range(38):
            nc.tensor.matmul(dps[:], dummy[:, :64], dummy[:, :128],
                             start=True, stop=True)

        # the first agent's input and the first shared-weight m-tile gate
        # the first matmul: load them before anything else, in k-subtile
        # pieces so the PE can start early
        ws_r = ws.rearrange("(ks p) h -> p ks h", p=P)
        a0 = agent_list[0][1]
        x0_first = xpool.tile([P, KD, NB], bf16, tag="x0")
        x0_first_r = x0t[a0].rearrange("(ks p) b -> p ks b", p=P)
        wsm = [const.tile([P, KD, P], bf16, tag=f"wsm{ms}", name=f"wsm{ms}")
               for ms in range(MH)]
        nc.sync.dma_start(x0_first[:, 0, :], x0_first_r[:, 0, :])
        nc.sync.dma_start(wsm[0][:, 0, :], ws_r[:, 0, 0:P])
        for ks in range(1, KD):
            nc.sync.dma_start(x0_first[:, ks, :], x0_first_r[:, ks, :])
            nc.sync.dma_start(wsm[0][:, ks, :], ws_r[:, ks, 0:P])
        for ms in range(1, MH):
            nc.sync.dma_start(wsm[ms][:], ws_r[:, :, ms * P:(ms + 1) * P])
        bs_t = const.tile([P, MH], f32)
        nc.sync.dma_start(bs_t[:], bs.rearrange("(ms p) -> p ms", p=P))

        x0_tiles = {0: x0_first}

        def dma_x0(t):
            if t >= NAG or t in x0_tiles:
                return
            a = agent_list[t][1]
            x0_t = xpool.tile([P, KD, NB], bf16, tag="x0", name=f"x0_{a}")
            # split per k-subtile: pieces spread over the HW DMA queues,
            # so one slow queue can't hold back the whole tile
            x0_r = x0t[a].rearrange("(ks p) b -> p ks b", p=P)
            for ks in range(KD):
                nc.sync.dma_start(x0_t[:, ks, :], x0_r[:, ks, :])
            x0_tiles[t] = x0_t

        def load_group_weights(s):
            w1_t = wpool.tile([P, KH, H], bf16, tag="w1", name=f"w1_{s}")
            w1_r = w1g[s].rearrange("(ks p) h -> p ks h", p=P)
            for ks in range(KH):
                nc.sync.dma_start(w1_t[:, ks, :], w1_r[:, ks, :])
            b1_t = wpool.tile([P, MH], f32, tag="b1", name=f"b1_{s}")
            nc.sync.dma_start(b1_t[:], b1g[s].rearrange("(ms p) -> p ms", p=P))
            w2_t = wpool.tile([P, KH, NA], bf16, tag="w2", name=f"w2_{s}")
            nc.sync.dma_start(
                w2_t[:], w2g[s].rearrange("(ks p) n -> p ks n", p=P))
            b2_t = wpool.tile([NA, 1], f32, tag="b2", name=f"b2_{s}")
            nc.sync.dma_start(b2_t[:], b2r[s])
            return (w1_t, b1_t, w2_t, b2_t)

        def emit_l1(a, x0_t):
            x1_t = x1pool.tile([P, MH, NB], bf16, tag="x1", name=f"x1_{a}")
            for ms in range(MH):
                ps1 = psum.tile([P, NB], f32, tag="ps", name=f"ps1_{a}_{ms}")
                for ks in range(KD):
                    nc.tensor.matmul(
                        ps1[:], wsm[ms][:, ks, :], x0_t[:, ks, :],
                        start=(ks == 0), stop=(ks == KD - 1),
                    )
                if ms % 2:
                    nc.vector.tensor_scalar(
                        x1_t[:, ms, :], ps1[:], bs_t[:, ms:ms + 1], 0.0,
                        mybir.AluOpType.add, mybir.AluOpType.max)
                else:
                    nc.scalar.activation(x1_t[:, ms, :], ps1[:], Relu,
                                         bias=bs_t[:, ms:ms + 1])
            return x1_t

        def emit_l3_k(a, k, h_t, w2_t, ps3):
            nc.tensor.matmul(
                ps3[:NA, :],
                w2_t[:, k, :],
                h_t[:, k, :],
                start=(k == 0), stop=(k == KH - 1),
                skip_group_check=True,
            )

        def emit_l2(a, x1_t, wt):
            w1_t, b1_t, w2_t, b2_t = wt
            h_t = hpool.tile([P, MH, NB], bf16, tag="h", name=f"h_{a}")
            ps3 = psum3.tile([P, NB], f32, tag="ps3", name=f"ps3_{a}")
            for ms in range(MH):
                ps2 = psum2.tile([P, NB], f32, tag="ps2", name=f"ps2_{a}_{ms}")
                for ks in range(KH):
                    nc.tensor.matmul(
                        ps2[:],
                        w1_t[:, ks, ms * P:(ms + 1) * P],
                        x1_t[:, ks, :],
                        start=(ks == 0), stop=(ks == KH - 1),
                    )
                # interleave L3's k-tile ms-1 matmul BEFORE emitting this
                # m-tile's activation: its h dependency (act of m-tile
                # ms-1) then has this whole m-tile's streaming as slack
                if ms >= 1:
                    emit_l3_k(a, ms - 1, h_t, w2_t, ps3)
                if ms % 2:
                    nc.vector.tensor_scalar(
                        h_t[:, ms, :], ps2[:], b1_t[:, ms:ms + 1], 0.0,
                        mybir.AluOpType.add, mybir.AluOpType.max)
                else:
                    nc.scalar.activation(h_t[:, ms, :], ps2[:], Relu,
                                         bias=b1_t[:, ms:ms + 1])
            return h_t, ps3

        def emit_l3_tail(a, h_t, wt, ps3):
            w1_t, b1_t, w2_t, b2_t = wt
            emit_l3_k(a, KH - 1, h_t, w2_t, ps3)
            o_t = opool.tile([NA, NB], f32, tag="o", name=f"o_{a}")
            nc.vector.tensor_add(
                o_t[:], ps3[:NA, :],
                b2_t[:NA, 0:1].to_broadcast((NA, NB)),
            )
            nc.sync.dma_start(yt[a], o_t[:])

        pend_l2 = None   # (a, x1_t, wt) — L1 done, L2 not yet emitted
        pend_tail = None  # (a, h_t, wt, ps3) — L2 done, L3 tail deferred
        cur_s = -1
        wt = None
        for t, (s, a) in enumerate(agent_list):
            if s != cur_s:
                wt = load_group_weights(s)
                cur_s = s
            if t == 0:
                dma_x0(1)
                dma_x0(2)
            dma_x0(t + 3)
            # two-stage software pipeline over agents:
            #   L1(a) | l3_tail(a-2) | L2+L3main(a-1)
            x1_t = emit_l1(a, x0_tiles.pop(t))
            if pend_tail is not None:
                emit_l3_tail(*pend_tail)
                pend_tail = None
            if pend_l2 is not None:
                pa, px1, pwt = pend_l2
                h_t, ps3 = emit_l2(pa, px1, pwt)
                pend_tail = (pa, h_t, pwt, ps3)
            pend_l2 = (a, x1_t, wt)
        # drain
        if pend_tail is not None:
            emit_l3_tail(*pend_tail)
        pa, px1, pwt = pend_l2
        h_t, ps3 = emit_l2(pa, px1, pwt)
        emit_l3_tail(pa, h_t, pwt, ps3)

    nc.compile()
    return nc


def kernel(x0, W_shared, b_shared, W1, b1, W2, b2, route,
           _trace=False, _tmpdir=None):
    import ml_dtypes
    from concourse.bass_utils import run_bass_kernel_spmd

    bf16 = ml_dtypes.bfloat16
    x0 = np.asarray(x0, dtype=np.float32)
    W_shared = np.asarray(W_shared, dtype=np.float32)
    b_shared = np.asarray(b_shared, dtype=np.float32)
    W1 = np.asarray(W1, dtype=np.float32)
    b1 = np.asarray(b1, dtype=np.float32)
    W2 = np.asarray(W2, dtype=np.float32)
    b2 = np.asarray(b2, dtype=np.float32)
    route = np.asarray(route)

    B, A, D = x0.shape
    H = W_shared.shape[1]
    NA = W2.shape[2]
    Bl = B // N_CORES
    JB = Bl // P

    experts, inv = np.unique(route, return_inverse=True)
    groups = tuple(tuple(np.where(inv == s)[0].tolist())
                   for s in range(len(experts)))

    key = (B, A, D, H, NA, groups)
    nc = _cache.get(key)
    if nc is None:
        nc = _build(A, D, H, NA, Bl, groups)
        _cache[key] = nc

    # host-side shard + transpose to feature-major, cast to bf16,
    # gather distinct experts
    x0t = np.ascontiguousarray(
        x0.astype(bf16).reshape(N_CORES, Bl, A, D).transpose(0, 2, 3, 1))
    w1g = np.ascontiguousarray(W1[experts].astype(bf16))
    b1g = np.ascontiguousarray(b1[experts])
    w2g = np.ascontiguousarray(W2[experts].astype(bf16))
    b2r = np.ascontiguousarray(b2[experts])[:, :, None]  # [E, NA, 1]
    ws_b = W_shared.astype(bf16)

    in_maps = [
        dict(x0t=x0t[c], ws=ws_b, bs=b_shared,
             w1g=w1g, b1g=b1g, w2g=w2g, b2r=b2r)
        for c in range(N_CORES)
    ]
    # the axon-proxied runtime occasionally reports a transient
    # "device unrecoverable" right after another process released the
    # cores; a short-delay retry recovers it
    import time
    last_err = None
    for attempt in range(3):
        try:
            res = run_bass_kernel_spmd(nc, in_maps,
                                       core_ids=list(range(N_CORES)),
                                       trace=_trace, tmpdir=_tmpdir)
            break
        except Exception as e:  # noqa: BLE001
            last_err = e
            time.sleep(5.0 * (attempt + 1))
    else:
        raise last_err
    kernel.last_exec_time_ns = res.exec_time_ns
    yt = np.stack([res.results[c]["yt"] for c in range(N_CORES)])  # [NC,A,NA,Bl]
    y = np.ascontiguousarray(yt.transpose(0, 3, 1, 2)).reshape(B, A, NA)
    return y
